# revision 31
# baseline (speedup 1.0000x reference)
"""Causal self-attention (B=2048, T=128, C=192, H=6, D=32) on 8 TRN2 cores.

Data-parallel over batch: 256 elems/core. v3 redesign vs v2 baseline:
  - fp32-source PE transposes fuse the x fp32->bf16 cast (no DVE CAST op)
  - k-bias dropped (softmax row-invariant); v-bias folded into proj bias
    on host; q-bias via ones-row in the contraction dim
  - softmax: batched exp on ScalarE; mask+rowsum fused per head via
    tensor_tensor_reduce on DVE; normalization deferred to the yT
    evacuation: scalar_tensor_tensor multiplies by a PE-built broadcast
    of 1/rowsum (tiny rrec transpose + head-expander matmuls)
  - PSUM: 6 independent tags = 8 banks exactly, no cross-stage strangles
  - evacs balanced: ScalarE gets qk/outs/rrecT, DVE gets xT/v/PT/yT
"""

import sys

sys.path.insert(0, "/opt/trn_rl_repo")

import numpy as np
import ml_dtypes

N_CORES = 8
B, T, C = 2048, 128, 192
NH, HD = 6, 32
BL = B // N_CORES  # 256 per core

_CACHE = {}


def _build(bl, stage=99):
    from contextlib import ExitStack

    import concourse.bass as bass
    import concourse.mybir as mybir
    import concourse.tile as tile
    from concourse import bacc

    fp32 = mybir.dt.float32
    bf16 = mybir.dt.bfloat16
    AF = mybir.ActivationFunctionType
    ALU = mybir.AluOpType

    nc = bacc.Bacc("TRN2", target_bir_lowering=False, debug=False)

    x_d = nc.dram_tensor("x", [bl, T, C], fp32, kind="ExternalInput")
    # qk weights: 4 M-tiles [q h0-3 | q h4-5 +pad | k h0-3 | k h4-5 +pad]
    wA_d = nc.dram_tensor("wA", [128, 512], bf16, kind="ExternalInput")
    wB_d = nc.dram_tensor("wB", [65, 512], bf16, kind="ExternalInput")
    # v weights separate (natural-orientation matmul)
    wvA_d = nc.dram_tensor("wvA", [128, 192], bf16, kind="ExternalInput")
    wvB_d = nc.dram_tensor("wvB", [64, 192], bf16, kind="ExternalInput")
    wpA_d = nc.dram_tensor("wpA", [128, 192], bf16, kind="ExternalInput")
    wpB_d = nc.dram_tensor("wpB", [65, 192], bf16, kind="ExternalInput")
    tril_d = nc.dram_tensor("tril", [128, 6, 128], bf16, kind="ExternalInput")
    ident_d = nc.dram_tensor("ident", [128, 128], bf16, kind="ExternalInput")
    identf_d = nc.dram_tensor("identf", [128, 128], fp32, kind="ExternalInput")
    e6a_d = nc.dram_tensor("e6a", [6, 128], bf16, kind="ExternalInput")
    e6b_d = nc.dram_tensor("e6b", [6, 128], bf16, kind="ExternalInput")
    out_d = nc.dram_tensor("out", [bl, T, C], fp32, kind="ExternalOutput")

    with tile.TileContext(nc) as tc, ExitStack() as ctx:
        consts = ctx.enter_context(tc.tile_pool(name="consts", bufs=1))
        sb = ctx.enter_context(tc.tile_pool(name="sb", bufs=3))
        ps = ctx.enter_context(
            tc.tile_pool(name="ps", bufs=1, space=bass.MemorySpace.PSUM)
        )

        def cload(name, shape, dtype, src):
            t = consts.tile(shape, dtype, tag=name)
            nc.sync.dma_start(t[:], src[:])
            return t

        wA = cload("wA", [128, 512], bf16, wA_d)
        wB = cload("wB", [65, 512], bf16, wB_d)
        wvA = cload("wvA", [128, 192], bf16, wvA_d)
        wvB = cload("wvB", [64, 192], bf16, wvB_d)
        wpA = cload("wpA", [128, 192], bf16, wpA_d)
        wpB = cload("wpB", [65, 192], bf16, wpB_d)
        tril = cload("tril", [128, 6, 128], bf16, tril_d)
        ident = cload("ident", [128, 128], bf16, ident_d)
        identf = cload("identf", [128, 128], fp32, identf_d)
        e6a = cload("e6a", [6, 128], bf16, e6a_d)
        e6b = cload("e6b", [6, 128], bf16, e6b_d)

        for p in range(bl // 2):
            # ---- outer stage: 2 elems at once ----
            xf = sb.tile([128, 2, 192], fp32, tag="xf")
            nc.sync.dma_start(
                xf[:], x_d[2 * p : 2 * p + 2].rearrange("e t c -> t e c")
            )

            # cast fp32->bf16, then PE transposes
            x16 = sb.tile([128, 2, 192], bf16, tag="x16")
            nc.vector.tensor_copy(x16[:], xf[:])
            xTp = ps.tile([128, 2, 2, 128], bf16, tag="xTp", name=f"xTp_{p}")
            for e in range(2):
                nc.tensor.transpose(xTp[:, e, 0, :], x16[:, e, 0:128], ident)
                nc.tensor.transpose(
                    xTp[0:64, e, 1, :], x16[:, e, 128:192], ident
                )
            xT = sb.tile([128, 2, 2, 128], bf16, tag="xT")
            nc.vector.tensor_copy(xT[:, :, 0, :], xTp[:, :, 0, :])
            nc.vector.tensor_copy(xT[0:64, :, 1, :], xTp[0:64, :, 1, :])
            nc.gpsimd.memset(xT[64:65, :, 1, :], 1.0)

            # qk^T = W^T x^T (4 M-tiles, bias via ones-row), both elems
            qkP = ps.tile([128, 4, 2, 128], fp32, tag="qs", name=f"qkP_{p}")
            for j in range(4):
                nc.tensor.matmul(
                    qkP[:, j, :, :],
                    wA[:, 128 * j : 128 * (j + 1)],
                    xT[:, :, 0, :],
                    start=True,
                    stop=False,
                )
                nc.tensor.matmul(
                    qkP[:, j, :, :],
                    wB[:, 128 * j : 128 * (j + 1)],
                    xT[0:65, :, 1, :],
                    start=False,
                    stop=True,
                )
            qk16 = sb.tile([128, 4, 2, 128], bf16, tag="qk16")
            nc.scalar.copy(qk16[:], qkP[:])

            # v = x Wv (natural [s, d]), per elem
            vP = ps.tile([128, 2, 192], fp32, tag="vP", name=f"vP_{p}")
            for e in range(2):
                nc.tensor.matmul(
                    vP[:, e, :], xT[:, e, 0, :], wvA[:], start=True, stop=False
                )
                nc.tensor.matmul(
                    vP[:, e, :],
                    xT[0:64, e, 1, :],
                    wvB[:],
                    start=False,
                    stop=True,
                )
            v16 = sb.tile([128, 2, 192], bf16, tag="v16")
            nc.vector.tensor_copy(v16[:], vP[:])

            if stage < 1:
                for e in range(2):
                    outs = sb.tile([128, 192], fp32, tag="outs")
                    nc.vector.tensor_copy(outs[:], vP[:, e, :])
                    nc.sync.dma_start(out_d[2 * p + e], outs[:])
                continue

            # ---- per-elem softmax + attention + proj ----
            for e in range(2):
                # S_h = q_h^T k_h; concurrent row-strip matmuls must land in
                # distinct PSUM banks: head h -> bank h%4, word-offset h//4
                S = ps.tile([128, 4, 512], fp32, tag="qs", name=f"S_{p}_{e}")
                for h in range(NH):
                    r = 32 * (h % 4)
                    jq = 0 if h < 4 else 1
                    c0 = 0 if h < 4 else 128
                    q = qk16[r : r + 32, jq, e, :]
                    k = qk16[r : r + 32, jq + 2, e, :]
                    nc.tensor.matmul(
                        S[:, h % 4, c0 : c0 + 128], q, k, start=True, stop=True,
                        tile_position=(r, 0),
                    )

                # P = exp(S), batched over heads
                P16 = sb.tile([128, 6, 128], bf16, tag="P16")
                nc.scalar.activation(P16[:, 0:4, :], S[:, :, 0:128], AF.Exp)
                nc.scalar.activation(P16[:, 4:6, :], S[:, 0:2, 128:256], AF.Exp)

                if stage < 2:
                    outs = sb.tile([128, 192], fp32, tag="outs")
                    nc.vector.tensor_copy(outs[:, 0:128], P16[:, 0, :])
                    nc.vector.tensor_copy(outs[:, 128:192], P16[:, 1, 0:64])
                    nc.sync.dma_start(out_d[2 * p + e], outs[:])
                    continue

                # mask (batched), then per-head rowsums
                Pm = sb.tile([128, 6, 128], bf16, tag="Pm")
                nc.vector.tensor_mul(Pm[:], P16[:], tril[:])
                rsum = sb.tile([128, 6], fp32, tag="rsum")
                nc.vector.reduce_sum(rsum[:], Pm[:], axis=mybir.AxisListType.X)

                # 1/rowsum, cast to bf16, transpose to row layout
                rrec = sb.tile([128, 6], fp32, tag="rrec")
                nc.vector.reciprocal(rrec[:], rsum[:])
                rrec16 = sb.tile([128, 6], bf16, tag="rrec16")
                nc.vector.tensor_copy(rrec16[:], rrec[:])

                if stage < 3:
                    outs = sb.tile([128, 192], fp32, tag="outs")
                    nc.vector.tensor_copy(outs[:, 0:128], Pm[:, 0, :])
                    nc.vector.memset(outs[:, 128:192], 0.0)
                    nc.sync.dma_start(out_d[2 * p + e], outs[:])
                    continue

                # PT tile: [:, 0:6, :] = P^T per head; [0:6, 6, :] = rrec^T
                PTp = ps.tile([128, 7, 128], bf16, tag="PTp", name=f"PT_{p}_{e}")
                for h in range(NH):
                    nc.tensor.transpose(PTp[:, h, :], Pm[:, h, :], ident)
                nc.tensor.transpose(PTp[0:6, 6, :], rrec16[:], ident)
                PT16 = sb.tile([128, 7, 128], bf16, tag="PT16")
                nc.vector.tensor_copy(PT16[:, 0:6, :], PTp[:, 0:6, :])
                nc.vector.tensor_copy(PT16[0:6, 6, :], PTp[0:6, 6, :])

                if stage < 4:
                    outs = sb.tile([128, 192], fp32, tag="outs")
                    nc.vector.tensor_copy(outs[:, 0:128], PT16[:, 0, :])
                    nc.vector.tensor_copy(outs[:, 128:192], PT16[:, 1, 0:64])
                    nc.sync.dma_start(out_d[2 * p + e], outs[:])
                    continue

                # yo tile: [0:256]=yt, [256:384]=rrecB h0-3, [384:512]=rrecB h4-5
                yo = ps.tile([128, 4, 128], fp32, tag="yo", name=f"yo_{p}_{e}")
                rrT = PT16[0:6, 6, :]
                nc.tensor.matmul(yo[:, 2, :], e6a[:], rrT, start=True, stop=True)
                nc.tensor.matmul(
                    yo[0:64, 3, :], e6b[:, 0:64], rrT, start=True, stop=True
                )
                for h in range(NH):
                    r = 32 * (h % 4)
                    j = 0 if h < 4 else 1
                    nc.tensor.matmul(
                        yo[r : r + 32, j, :],
                        v16[:, e, 32 * h : 32 * h + 32],
                        PT16[:, h, :],
                        start=True,
                        stop=True,
                        tile_position=(0, r),
                    )

                # yT evac fused with normalization: yT = yt * rrecB
                # (DVE may read only one PSUM input: stage rrecB to SBUF)
                rrB = sb.tile([128, 2, 128], bf16, tag="rrB")
                nc.vector.tensor_copy(rrB[:, 0, :], yo[:, 2, :])
                nc.vector.tensor_copy(rrB[0:64, 1, :], yo[0:64, 3, :])
                yT = sb.tile([128, 2, 128], bf16, tag="yT")
                nc.vector.scalar_tensor_tensor(
                    yT[:, 0, :],
                    yo[:, 0, :],
                    1.0,
                    rrB[:, 0, :],
                    ALU.mult,
                    ALU.mult,
                )
                nc.vector.scalar_tensor_tensor(
                    yT[0:64, 1, :],
                    yo[0:64, 1, :],
                    1.0,
                    rrB[0:64, 1, :],
                    ALU.mult,
                    ALU.mult,
                )
                nc.gpsimd.memset(yT[64:65, 1, :], 1.0)

                if stage < 5:
                    outs = sb.tile([128, 192], fp32, tag="outs")
                    nc.vector.tensor_copy(outs[:, 0:128], yT[:, 0, :])
                    nc.vector.memset(outs[:, 128:192], 0.0)
                    nc.sync.dma_start(out_d[2 * p + e], outs[:])
                    continue

                # proj: out = yT^T Wp (+bias via ones-row); reuse yo[0:192]
                outp = yo[:].rearrange("p a b -> p (a b)")[:, 0:192]
                nc.tensor.matmul(outp, yT[:, 0, :], wpA[:], start=True, stop=False)
                nc.tensor.matmul(
                    outp, yT[0:65, 1, :], wpB[:], start=False, stop=True
                )
                outs = sb.tile([128, 192], fp32, tag="outs")
                nc.scalar.copy(outs[:], outp)
                nc.sync.dma_start(out_d[2 * p + e], outs[:])

    nc.finalize()
    return nc


def _prep_inputs(x, w_qkv, b_qkv, w_proj, b_proj, bl):
    bf = ml_dtypes.bfloat16
    scale = 1.0 / np.sqrt(HD)
    w = np.array(w_qkv, dtype=np.float32, copy=True)
    b = np.array(b_qkv, dtype=np.float32, copy=True)
    wp = np.asarray(w_proj, dtype=np.float32)
    bp = np.asarray(b_proj, dtype=np.float32)

    # fold 1/sqrt(d) into q weights+bias; drop k bias (softmax row-invariant);
    # fold v bias into the proj bias
    wq = w[:, 0:192] * scale
    bq = b[0:192] * scale
    wk = w[:, 192:384]
    wv = w[:, 384:576]
    bv = b[384:576]
    b_eff = bp + bv @ wp

    # qk col-pack: [q h0-3 | q h4-5 + pad64 | k h0-3 | k h4-5 + pad64]
    z64 = np.zeros((192, 64), np.float32)
    qk = np.concatenate(
        [wq[:, 0:128], wq[:, 128:192], z64, wk[:, 0:128], wk[:, 128:192], z64],
        axis=1,
    )  # [192, 512]
    brow = np.zeros((1, 512), np.float32)
    brow[0, 0:128] = bq[0:128]
    brow[0, 128:192] = bq[128:192]
    wA = qk[0:128].astype(bf)
    wB = np.concatenate([qk[128:192], brow], axis=0).astype(bf)

    wvA = wv[0:128].astype(bf)
    wvB = wv[128:192].astype(bf)
    wpA = wp[0:128].astype(bf)
    wpB = np.concatenate([wp[128:192], b_eff[None, :]], axis=0).astype(bf)

    tril = np.ascontiguousarray(
        np.broadcast_to(
            np.tril(np.ones((128, 128), np.float32)), (6, 128, 128)
        ).transpose(1, 0, 2)
    ).astype(bf)
    ident = np.eye(128, dtype=np.float32)
    e6a = np.zeros((6, 128), np.float32)
    e6b = np.zeros((6, 128), np.float32)
    for h in range(4):
        e6a[h, 32 * h : 32 * h + 32] = 1.0
    for h in range(4, 6):
        e6b[h, 32 * (h - 4) : 32 * (h - 4) + 32] = 1.0

    xs = np.ascontiguousarray(np.asarray(x, dtype=np.float32)).reshape(
        -1, bl, T, C
    )
    base = {
        "wA": wA,
        "wB": wB,
        "wvA": wvA,
        "wvB": wvB,
        "wpA": wpA,
        "wpB": wpB,
        "tril": tril,
        "ident": ident.astype(bf),
        "identf": ident,
        "e6a": e6a.astype(bf),
        "e6b": e6b.astype(bf),
    }
    return [dict(base, x=xs[i]) for i in range(xs.shape[0])]


def _run(x, w_qkv, b_qkv, w_proj, b_proj, bl=BL, n_cores=N_CORES, trace=False):
    from concourse.bass_utils import run_bass_kernel_spmd

    key = bl
    if key not in _CACHE:
        _CACHE[key] = _build(bl)
    nc = _CACHE[key]
    maps = _prep_inputs(x, w_qkv, b_qkv, w_proj, b_proj, bl)[:n_cores]
    res = run_bass_kernel_spmd(
        nc, maps, core_ids=list(range(len(maps))), trace=trace
    )
    out = np.concatenate([r["out"] for r in res.results], axis=0)
    return out, res


def kernel(x, w_qkv, b_qkv, w_proj, b_proj):
    out, _ = _run(x, w_qkv, b_qkv, w_proj, b_proj)
    return out.reshape(B, T, C).astype(np.float32)
